# revision 9
# baseline (speedup 1.0000x reference)
"""GCN (3-layer message passing) Trainium2 Bass kernel, 8-way node-sharded.

v2 strategy (vs v1: fp32 single-row gather, 128-wide one-hots, DVE scaling):
  - Feature table in bf16. dma_gather requires 256B-multiple elements, so
    rows are gathered in PAIRS (256B = 2 bf16 rows, index = padded_row >> 1).
    Pair indices span [0, 25088) < 2^15, so one int16 index space (no table
    halves).
  - Dest rows blocked 64-wide: halves the one-hot build work on DVE (the
    broadcast is_equal build gets no 16-bit speedup, so width is the lever).
  - Per 128-edge slot, TWO bf16 one-hot matmuls (even/odd pair parity pick
    the correct 64-col half of the gathered pair) at 1 cycle/row.
  - The "+ xs" self-term folds into PSUM via an identity matmul; the final
    norm/norm^2 scaling runs on the otherwise-idle Activation engine
    (Copy with per-partition scale), leaving DVE with only one-hot builds.
  - Per layer: AllGather bf16 table (half of v1 traffic) -> per-edge pair
    gather -> one-hot matmuls into [64,64] PSUM accumulators -> ACT scale.

Self-contained: hardcodes the problem shapes; only needs numpy + the
concourse stack at /opt/trn_rl_repo.
"""

import sys

for _p in ("/opt/trn_rl_repo",):
    if _p not in sys.path:
        sys.path.insert(0, _p)

from dataclasses import dataclass, field

import numpy as np
import ml_dtypes

BF16 = ml_dtypes.bfloat16


@dataclass
class Cfg:
    N: int = 50000
    IN: int = 128
    HID: int = 64
    LAYERS: int = 3
    CORES: int = 8
    BW: int = 64       # dest block width (one-hot free dim)
    GB: int = 8        # 64-blocks per processing group
    MAXC: int = 12     # max chunks (x128 idxs) per dma_gather call
    GBUFS: int = 2     # gather-tile double/triple buffering
    OBUFS: int = 2     # one-hot tile buffering
    SCRATCH: int = 16384  # dynamic dma scratch (swdge ring) bytes/partition
    SP: bool = False   # dma_gather single_packet mode
    AGCHUNK: int = 4   # split each AllGather into this many row-chunks
    REPEAT: int = 1    # repeat the layer stack (timing experiments only)

    @property
    def NPC(self):  # nodes per core
        assert self.N % self.CORES == 0
        return self.N // self.CORES

    @property
    def BPC(self):  # BW-row blocks per core
        return (self.NPC + self.BW - 1) // self.BW

    @property
    def NPAD(self):
        return self.BPC * self.BW

    @property
    def NGROUPS(self):
        return (self.BPC + self.GB - 1) // self.GB

    def group_blocks(self, g):
        return list(range(g * self.GB, min((g + 1) * self.GB, self.BPC)))

    def ag_chunks(self):
        """[(row0, row1, end_group)] per AG chunk; chunk-major table layout."""
        splits = [a for a in np.array_split(np.arange(self.NGROUPS), self.AGCHUNK) if len(a)]
        out = []
        for a in splits:
            r0 = int(a[0]) * self.GB * self.BW
            r1 = min((int(a[-1]) + 1) * self.GB * self.BW, self.NPAD)
            out.append((r0, r1, int(a[-1])))
        return out


@dataclass
class Sched:
    """Per-block slot counts (slots = 128-edge chunks), shared across cores."""
    slots: np.ndarray  # [BPC] int

    slot_block: list = field(default_factory=list)       # slot -> block
    group_call_slots: list = field(default_factory=list)  # [g] -> (s0, s1)
    block_slot_ranges: list = field(default_factory=list)  # [b] -> (s0, s1)
    total: int = 0

    def finalize(self, cfg: Cfg):
        self.slot_block = []
        self.group_call_slots = []
        self.block_slot_ranges = [None] * cfg.BPC
        s = 0
        for g in range(cfg.NGROUPS):
            s0g = s
            for b in cfg.group_blocks(g):
                bs0 = s
                for _ in range(int(self.slots[b])):
                    self.slot_block.append(b)
                    s += 1
                self.block_slot_ranges[b] = (bs0, s)
            self.group_call_slots.append((s0g, s))
        self.total = s


def make_schedule(edge_index: np.ndarray, cfg: Cfg):
    """Compute the shared slot schedule + per-core slot contents."""
    row = np.asarray(edge_index[0], dtype=np.int64)
    col = np.asarray(edge_index[1], dtype=np.int64)

    core = row // cfg.NPC
    rloc = row % cfg.NPC
    blk = rloc // cfg.BW
    rrel = (rloc % cfg.BW).astype(np.float32)
    # column ids live in the PADDED, AG-chunk-major table: chunk -> core ->
    # local pair row. With AGCHUNK=1 this is plain core-major.
    ccore = col // cfg.NPC
    clocal = col % cfg.NPC
    r = clocal >> 1                        # local pair row < NPAD/2
    par = (clocal & 1).astype(np.int64)    # parity within the pair
    chunks = cfg.ag_chunks()
    cbp = np.array([c[0] // 2 for c in chunks] + [cfg.NPAD // 2])  # pair bounds
    w = cbp[1:] - cbp[:-1]                 # chunk widths (pairs)
    off8 = np.concatenate([[0], np.cumsum(cfg.CORES * w)])
    ck = np.searchsorted(cbp, r, side="right") - 1
    pidx = (off8[ck] + ccore * w[ck] + (r - cbp[ck])).astype(np.int16)

    # group edges by (core, block); sort by pair id within each segment so
    # gather descriptors walk ascending HBM addresses
    key = core * cfg.BPC + blk
    order = np.lexsort((pidx, key))
    key_s = key[order]
    rrel_s = rrel[order]
    pidx_s = pidx[order]
    par_s = par[order]

    bounds = np.searchsorted(
        key_s, np.arange(cfg.CORES * cfg.BPC + 1), side="left"
    )
    counts = (bounds[1:] - bounds[:-1]).reshape(cfg.CORES, cfg.BPC)

    slots = np.max((counts + 127) // 128, axis=0)  # [BPC]

    sched = Sched(slots=slots)
    sched.finalize(cfg)

    TC = sched.total
    per_core = []
    NEG = np.float32(-100.0)
    for c in range(cfg.CORES):
        idx_flat = np.zeros((TC, 128), dtype=np.int16)
        rrE_flat = np.full((TC, 128), NEG, dtype=np.float32)
        rrO_flat = np.full((TC, 128), NEG, dtype=np.float32)
        for b in range(cfg.BPC):
            s0, s1 = sched.block_slot_ranges[b]
            cap = (s1 - s0) * 128
            k = c * cfg.BPC + b
            e0, e1 = bounds[k], bounds[k + 1]
            n = e1 - e0
            assert n <= cap, (c, b, n, cap)
            if cap == 0:
                continue
            ci = np.zeros(cap, dtype=np.int16)
            rE = np.full(cap, NEG, dtype=np.float32)
            rO = np.full(cap, NEG, dtype=np.float32)
            ci[:n] = pidx_s[e0:e1]
            pe = par_s[e0:e1] == 0
            rE[:n] = np.where(pe, rrel_s[e0:e1], NEG)
            rO[:n] = np.where(~pe, rrel_s[e0:e1], NEG)
            idx_flat[s0:s1] = ci.reshape(-1, 128)
            rrE_flat[s0:s1] = rE.reshape(-1, 128)
            rrO_flat[s0:s1] = rO.reshape(-1, 128)

        # gather index tile layout: [128, TC*8] int16; logical edge k of
        # slot s lives at [k % 16, s*8 + k // 16], replicated across the
        # eight 16-partition groups.
        idx_tile = np.zeros((128, TC * 8), dtype=np.int16)
        base = idx_flat.reshape(TC, 8, 16).transpose(2, 0, 1).reshape(16, TC * 8)
        for rep in range(8):
            idx_tile[rep * 16:(rep + 1) * 16] = base

        # rr tiles: [128, TC]; partition = edge position in slot.
        rrE_tile = rrE_flat.T.astype(BF16)
        rrO_tile = rrO_flat.T.astype(BF16)

        per_core.append((idx_tile, rrE_tile, rrO_tile))

    return sched, per_core


def host_inputs(x, edge_index, W, b, cfg: Cfg):
    """Build per-core in_maps (numpy only)."""
    x = np.asarray(x, dtype=np.float32)
    W = np.asarray(W, dtype=np.float32)
    b = np.asarray(b, dtype=np.float32)

    sched, per_core = make_schedule(edge_index, cfg)

    row = np.asarray(edge_index[0], dtype=np.int64)
    deg = np.bincount(row, minlength=cfg.N).astype(np.float32)
    norm = 1.0 / np.sqrt(1.0 + deg)

    iota64 = np.tile(np.arange(cfg.BW, dtype=np.float32), (128, 1)).astype(BF16)
    I64 = np.eye(cfg.BW, dtype=np.float32).astype(BF16)

    in_maps = []
    for c in range(cfg.CORES):
        r0 = c * cfg.NPC
        xT = np.zeros((cfg.IN, cfg.NPAD), dtype=np.float32)
        xT[:, : cfg.NPC] = x[r0 : r0 + cfg.NPC].T
        nc_ = np.ones(cfg.NPAD, dtype=np.float32)
        nc_[: cfg.NPC] = norm[r0 : r0 + cfg.NPC]
        norm64 = nc_.reshape(cfg.BPC, cfg.BW).T.copy()        # [64, BPC]
        norm264 = (norm64 * norm64).copy()
        idx_tile, rrE_tile, rrO_tile = per_core[c]
        in_maps.append(
            {
                "xT": xT,
                "Wm": W.copy(),
                "bvals": b.reshape(1, cfg.HID).copy(),
                "iota64": iota64.copy(),
                "I64": I64.copy(),
                "norm64": norm64,
                "norm264": norm264,
                "idx_all": idx_tile,
                "rrE_all": rrE_tile,
                "rrO_all": rrO_tile,
            }
        )
    return sched, in_maps


def build_bass(cfg: Cfg, sched: Sched, no_ag: bool = False, ablate=()):
    """Emit the Tile program. Returns compiled nc."""
    from concourse import bacc, bass, mybir, tile

    f32 = mybir.dt.float32
    bf16 = mybir.dt.bfloat16
    i16 = mybir.dt.int16
    EQ = mybir.AluOpType.is_equal
    COPY = mybir.ActivationFunctionType.Copy

    BW = cfg.BW
    TC = sched.total
    nc = bacc.Bacc(
        "TRN2",
        target_bir_lowering=False,
        debug=False,
        num_devices=cfg.CORES,
        num_swdge_queues=4,
        dynamic_dma_scratch_size=cfg.SCRATCH,
    )

    # ---- I/O ----
    xT_d = nc.dram_tensor("xT", [cfg.IN, cfg.NPAD], f32, kind="ExternalInput")
    W_d = nc.dram_tensor("Wm", [cfg.IN, cfg.HID], f32, kind="ExternalInput")
    b_d = nc.dram_tensor("bvals", [1, cfg.HID], f32, kind="ExternalInput")
    iota_d = nc.dram_tensor("iota64", [128, BW], bf16, kind="ExternalInput")
    I64_d = nc.dram_tensor("I64", [BW, BW], bf16, kind="ExternalInput")
    n1_d = nc.dram_tensor("norm64", [BW, cfg.BPC], f32, kind="ExternalInput")
    n2_d = nc.dram_tensor("norm264", [BW, cfg.BPC], f32, kind="ExternalInput")
    idx_d = nc.dram_tensor("idx_all", [128, TC * 8], i16, kind="ExternalInput")
    rrE_d = nc.dram_tensor("rrE_all", [128, TC], bf16, kind="ExternalInput")
    rrO_d = nc.dram_tensor("rrO_all", [128, TC], bf16, kind="ExternalInput")
    out_d = nc.dram_tensor("out", [cfg.NPAD, cfg.HID], f32, kind="ExternalOutput")

    # internal DRAM: AG input (own xs rows) + AG output (paired full table)
    xs_in = [
        nc.dram_tensor(f"xs_in{l}", [cfg.NPAD, cfg.HID], bf16)
        for l in range(cfg.LAYERS)
    ]
    _aspace = "Shared" if cfg.CORES > 4 else "Local"
    NPAIR = cfg.CORES * cfg.NPAD // 2  # 25088 paired rows of 128 bf16
    xs_full = [
        nc.dram_tensor(
            f"xs_full{l}", [NPAIR, 2 * cfg.HID], bf16, addr_space=_aspace
        )
        for l in range(cfg.LAYERS)
    ]

    rg = [list(range(cfg.CORES))]

    with tile.TileContext(nc) as tc:
        with (
            tc.tile_pool(name="const", bufs=1) as constp,
            tc.tile_pool(name="gbuf", bufs=cfg.GBUFS) as gpool,
            tc.tile_pool(name="onehot", bufs=cfg.OBUFS) as opool,
            tc.tile_pool(name="xtp", bufs=2) as xtp,
            tc.tile_pool(name="res", bufs=2) as resp,
            tc.tile_pool(name="psum", bufs=8, space="PSUM") as psp,
        ):
            # ---- persistent SBUF ----
            W_s = constp.tile([cfg.IN, cfg.HID], f32, tag="W")
            b_s = constp.tile([1, cfg.HID], f32, tag="b")
            ones_s = constp.tile([1, cfg.HID], f32, tag="ones")
            iota_s = constp.tile([128, BW], bf16, tag="iota")
            I64_s = constp.tile([BW, BW], bf16, tag="I64")
            n1_s = constp.tile([BW, cfg.BPC], f32, tag="n1")
            n2_s = constp.tile([BW, cfg.BPC], f32, tag="n2")
            idx_s = constp.tile([128, TC * 8], i16, tag="idx")
            rrE_s = constp.tile([128, TC], bf16, tag="rrE")
            rrO_s = constp.tile([128, TC], bf16, tag="rrO")
            xs_ping = constp.tile([BW, cfg.BPC, cfg.HID], bf16, tag="xsA")
            xs_pong = constp.tile([BW, cfg.BPC, cfg.HID], bf16, tag="xsB")

            nc.sync.dma_start(W_s[:], W_d[:, :])
            nc.sync.dma_start(b_s[:], b_d[:, :])
            nc.sync.dma_start(iota_s[:], iota_d[:, :])
            nc.sync.dma_start(I64_s[:], I64_d[:, :])
            nc.sync.dma_start(n1_s[:], n1_d[:, :])
            nc.sync.dma_start(n2_s[:], n2_d[:, :])
            nc.sync.dma_start(idx_s[:], idx_d[:, :])
            nc.sync.dma_start(rrE_s[:], rrE_d[:, :])
            nc.sync.dma_start(rrO_s[:], rrO_d[:, :])
            nc.vector.memset(ones_s[:], 1.0)

            def store_group_to_dram(dram, g, src_ap):
                """src_ap [BW, nb, HID] -> dram rows [BW*b0, BW*b1)."""
                blocks = cfg.group_blocks(g)
                b0, b1 = blocks[0], blocks[-1] + 1
                dst = dram[BW * b0 : BW * b1, :].rearrange(
                    "(b p) h -> p b h", p=BW
                )
                nc.sync.dma_start(dst, src_ap)

            # ---- prologue: h0 = x @ W + b ; xs0 = norm * h0 ----
            for g in range(cfg.NGROUPS):
                blocks = cfg.group_blocks(g)
                nb = len(blocks)
                b0 = blocks[0]
                xtg = xtp.tile([cfg.IN, nb * BW], f32, tag="xtg")
                nc.sync.dma_start(
                    xtg[:], xT_d[:, BW * b0 : BW * (b0 + nb)]
                )
                for j, b in enumerate(blocks):
                    ps = psp.tile([BW, cfg.HID], f32, tag="ps")
                    nc.tensor.matmul(
                        ps[:], ones_s[:], b_s[:], start=True, stop=False
                    )
                    nc.tensor.matmul(
                        ps[:],
                        xtg[:, BW * j : BW * (j + 1)],
                        W_s[:],
                        start=False,
                        stop=True,
                    )
                    nc.scalar.activation(
                        xs_ping[:, b, :], ps[:], COPY,
                        scale=n1_s[:, b : b + 1],
                    )
                store_group_to_dram(xs_in[0], g, xs_ping[:, b0 : b0 + nb, :])

            # AG chunk boundaries: contiguous group ranges, each ending at a
            # group boundary so a chunk can launch as soon as its groups store.
            _chunks = cfg.ag_chunks()
            _chunk_end_group = {eg: i for i, (_r0, _r1, eg) in enumerate(_chunks)}
            _off8 = np.concatenate(
                [[0], np.cumsum([cfg.CORES * (r1 - r0) // 2 for r0, r1, _ in _chunks])]
            ).astype(int)

            def do_ag(l, k=None):
                if "tinyag" in ablate:
                    if k not in (None, 0):
                        return
                    nc.gpsimd.collective_compute(
                        "AllGather", mybir.AluOpType.bypass, replica_groups=rg,
                        ins=[xs_in[l][0:1, 0:8]], outs=[xs_full[l][0:1, 0:64]],
                    )
                    return
                if k is None:
                    ks = range(len(_chunks))
                else:
                    ks = [k]
                for kk in ks:
                    r0, r1, _eg = _chunks[kk]
                    o0 = _off8[kk]
                    o1 = o0 + cfg.CORES * (r1 - r0) // 2
                    nc.gpsimd.collective_compute(
                        "AllGather", mybir.AluOpType.bypass, replica_groups=rg,
                        ins=[xs_in[l][r0:r1, :]],
                        outs=[xs_full[l][o0:o1, :]],
                    )

            if not no_ag:
                do_ag(0)

            # ---- layers ----
            xs_cur, xs_nxt = xs_ping, xs_pong
            for _rep in range(cfg.REPEAT):
              for l in range(cfg.LAYERS):
                last = l == cfg.LAYERS - 1
                table = xs_full[l]
                qn = 0
                for g in range(cfg.NGROUPS):
                    blocks = cfg.group_blocks(g)
                    nb = len(blocks)
                    b0 = blocks[0]
                    s0, s1 = sched.group_call_slots[g]
                    nch = s1 - s0
                    G = None
                    ohE = ohO = None
                    if nch > 0:
                        G = gpool.tile([128, nch, 2 * cfg.HID], bf16, tag="G")
                        _nch_g = 1 if "gather" in ablate else nch
                        for o0 in range(0, _nch_g, cfg.MAXC):
                            o1 = min(o0 + cfg.MAXC, _nch_g)
                            n = o1 - o0
                            nc.gpsimd.dma_gather(
                                G[:, o0:o1, :],
                                table[:, :],
                                idx_s[:, 8 * (s0 + o0) : 8 * (s0 + o1)],
                                n * 128,
                                n * 128,
                                2 * cfg.HID,
                                single_packet=cfg.SP,
                                queue_num=qn % 4,
                            )
                            qn += 1
                        ohE = opool.tile([128, nch, BW], bf16, tag="ohE")
                        ohO = opool.tile([128, nch, BW], bf16, tag="ohO")
                        if "oh" in ablate:
                            nc.vector.memset(ohE[:], 0.0)
                            nc.vector.memset(ohO[:], 0.0)
                        else:
                            io_b = (
                                iota_s[:, :]
                                .unsqueeze(1)
                                .broadcast_to([128, nch, BW])
                            )
                            rE_b = (
                                rrE_s[:, s0:s1]
                                .unsqueeze(2)
                                .broadcast_to([128, nch, BW])
                            )
                            rO_b = (
                                rrO_s[:, s0:s1]
                                .unsqueeze(2)
                                .broadcast_to([128, nch, BW])
                            )
                            nc.vector.tensor_tensor(ohE[:], io_b, rE_b, EQ)
                            nc.vector.tensor_tensor(ohO[:], io_b, rO_b, EQ)

                    rtile = None
                    if last:
                        rtile = resp.tile([BW, nb, cfg.HID], f32, tag="r")
                    for j, b in enumerate(blocks):
                        ps = psp.tile([BW, cfg.HID], f32, tag="ps")
                        bs0, bs1 = sched.block_slot_ranges[b]
                        k = 0
                        if "mm" not in ablate:
                            for s in range(bs0 - s0, bs1 - s0):
                                nc.tensor.matmul(
                                    ps[:],
                                    ohE[:, s, :],
                                    G[:, s, 0 : cfg.HID],
                                    start=(k == 0),
                                    stop=False,
                                )
                                k += 1
                                nc.tensor.matmul(
                                    ps[:],
                                    ohO[:, s, :],
                                    G[:, s, cfg.HID : 2 * cfg.HID],
                                    start=False,
                                    stop=False,
                                )
                                k += 1
                        # self-term: ps += I @ xs_cur[b]
                        nc.tensor.matmul(
                            ps[:],
                            I64_s[:],
                            xs_cur[:, b, :],
                            start=(k == 0),
                            stop=True,
                        )
                        nsrc = n1_s if last else n2_s
                        dst = rtile[:, j, :] if last else xs_nxt[:, b, :]
                        nc.scalar.activation(
                            dst, ps[:], COPY, scale=nsrc[:, b : b + 1]
                        )
                    if last:
                        store_group_to_dram(out_d, g, rtile[:])
                    else:
                        store_group_to_dram(
                            xs_in[l + 1], g, xs_nxt[:, b0 : b0 + nb, :]
                        )
                        if not no_ag and g in _chunk_end_group:
                            do_ag(l + 1, _chunk_end_group[g])

                if not last:
                    xs_cur, xs_nxt = xs_nxt, xs_cur

    nc.compile()
    return nc


def bench_bass(nc, in_maps, n_cores, iters=20, warmup=2):
    """Repeat-execution device benchmark (no NTFF tracing in this container).

    Mirrors bass2jax.run_bass_via_pjrt's multi-core path, minus output-buffer
    donation so the compiled executable can be re-invoked. Returns
    (results_list, per_iter_seconds).
    """
    import time

    import jax
    from jax.experimental.shard_map import shard_map
    from jax.sharding import Mesh, NamedSharding, PartitionSpec

    from concourse import bass2jax, mybir

    bass2jax.install_neuronx_cc_hook()

    partition_name = (
        nc.partition_id_tensor.name if nc.partition_id_tensor else None
    )
    in_names, out_names, out_avals, zero_outs = [], [], [], []
    for alloc in nc.m.functions[0].allocations:
        if not isinstance(alloc, mybir.MemoryLocationSet):
            continue
        name = alloc.memorylocations[0].name
        if alloc.kind == "ExternalInput":
            if name != partition_name:
                in_names.append(name)
        elif alloc.kind == "ExternalOutput":
            out_names.append(name)
            shape = tuple(alloc.tensor_shape)
            dtype = mybir.dt.np(alloc.dtype)
            out_avals.append(jax.core.ShapedArray(shape, dtype))
            zero_outs.append(np.zeros(shape, dtype))
    n_params = len(in_names)
    all_names = in_names + out_names
    if partition_name is not None:
        all_names = all_names + [partition_name]

    def _body(*args):
        operands = list(args)
        if partition_name is not None:
            operands.append(bass2jax.partition_id_tensor())
        outs = bass2jax._bass_exec_p.bind(
            *operands,
            out_avals=tuple(out_avals),
            in_names=tuple(all_names),
            out_names=tuple(out_names),
            lowering_input_output_aliases=(),
            sim_require_finite=True,
            sim_require_nnan=True,
            nc=nc,
        )
        return tuple(outs)

    devices = jax.devices()[:n_cores]
    mesh = Mesh(np.asarray(devices), ("core",))
    spec = PartitionSpec("core")
    nin = n_params + len(zero_outs)
    sharded = jax.jit(
        shard_map(
            _body,
            mesh=mesh,
            in_specs=(spec,) * nin,
            out_specs=(spec,) * len(out_names),
            check_rep=False,
        ),
        keep_unused=True,
    )
    sh = NamedSharding(mesh, spec)
    args = [
        jax.device_put(
            np.concatenate([np.asarray(m[name]) for m in in_maps], axis=0), sh
        )
        for name in in_names
    ] + [
        jax.device_put(
            np.zeros((n_cores * z.shape[0], *z.shape[1:]), z.dtype), sh
        )
        for z in zero_outs
    ]

    out_arrs = None
    for _ in range(warmup):
        out_arrs = sharded(*args)
        jax.block_until_ready(out_arrs)
    t0 = time.perf_counter()
    for _ in range(iters):
        out_arrs = sharded(*args)
    jax.block_until_ready(out_arrs)
    t1 = time.perf_counter()

    results = [
        {
            name: np.asarray(out_arrs[i]).reshape(n_cores, *out_avals[i].shape)[c]
            for i, name in enumerate(out_names)
        }
        for c in range(n_cores)
    ]
    return results, (t1 - t0) / iters


def kernel(x, edge_index, W, b, cfg: Cfg | None = None, trace: bool = False):
    """Full-input entry point. Returns [N, HID] float32 (+ results if trace)."""
    cfg = cfg or Cfg()
    sched, in_maps = host_inputs(x, edge_index, W, b, cfg)
    nc = build_bass(cfg, sched)

    from concourse import bass_utils

    res = bass_utils.run_bass_kernel_spmd(
        nc,
        in_maps,
        core_ids=list(range(cfg.CORES)),
        trace=False,
    )
    out = np.concatenate(
        [r["out"][: cfg.NPC] for r in res.results], axis=0
    )
    if trace:
        return out, res
    return out


if __name__ == "__main__":
    pass


# revision 13
# speedup vs baseline: 1.0510x; 1.0510x over previous
"""GCN (3-layer message passing) Trainium2 Bass kernel, 8-way node-sharded.

v2 strategy (vs v1: fp32 single-row gather, 128-wide one-hots, DVE scaling):
  - Feature table in bf16. dma_gather requires 256B-multiple elements, so
    rows are gathered in PAIRS (256B = 2 bf16 rows, index = padded_row >> 1).
    Pair indices span [0, 25088) < 2^15, so one int16 index space (no table
    halves).
  - Dest rows blocked 64-wide: halves the one-hot build work on DVE (the
    broadcast is_equal build gets no 16-bit speedup, so width is the lever).
  - Per 128-edge slot, TWO bf16 one-hot matmuls (even/odd pair parity pick
    the correct 64-col half of the gathered pair) at 1 cycle/row.
  - The "+ xs" self-term folds into PSUM via an identity matmul; the final
    norm/norm^2 scaling runs on the otherwise-idle Activation engine
    (Copy with per-partition scale), leaving DVE with only one-hot builds.
  - Per layer: AllGather bf16 table (half of v1 traffic) -> per-edge pair
    gather -> one-hot matmuls into [64,64] PSUM accumulators -> ACT scale.

Self-contained: hardcodes the problem shapes; only needs numpy + the
concourse stack at /opt/trn_rl_repo.
"""

import sys

for _p in ("/opt/trn_rl_repo",):
    if _p not in sys.path:
        sys.path.insert(0, _p)

from dataclasses import dataclass, field

import numpy as np
import ml_dtypes

BF16 = ml_dtypes.bfloat16


@dataclass
class Cfg:
    N: int = 50000
    IN: int = 128
    HID: int = 64
    LAYERS: int = 3
    CORES: int = 8
    BW: int = 64       # dest block width (one-hot free dim)
    GB: int = 8        # 64-blocks per processing group
    MAXC: int = 12     # max chunks (x128 idxs) per dma_gather call
    GBUFS: int = 2     # gather-tile double/triple buffering
    OBUFS: int = 2     # one-hot tile buffering
    SCRATCH: int = 16384  # dynamic dma scratch (swdge ring) bytes/partition
    SP: bool = False   # dma_gather single_packet mode
    AGCHUNK: int = 4   # split each AllGather into this many row-chunks
    REPEAT: int = 1    # repeat the layer stack (timing experiments only)

    @property
    def NPC(self):  # nodes per core
        assert self.N % self.CORES == 0
        return self.N // self.CORES

    @property
    def BPC(self):  # BW-row blocks per core
        return (self.NPC + self.BW - 1) // self.BW

    @property
    def NPAD(self):
        return self.BPC * self.BW

    @property
    def NGROUPS(self):
        return (self.BPC + self.GB - 1) // self.GB

    def group_blocks(self, g):
        return list(range(g * self.GB, min((g + 1) * self.GB, self.BPC)))

    def ag_chunks(self):
        """[(row0, row1, end_group)] per AG chunk; chunk-major table layout."""
        splits = [a for a in np.array_split(np.arange(self.NGROUPS), self.AGCHUNK) if len(a)]
        out = []
        for a in splits:
            r0 = int(a[0]) * self.GB * self.BW
            r1 = min((int(a[-1]) + 1) * self.GB * self.BW, self.NPAD)
            out.append((r0, r1, int(a[-1])))
        return out


@dataclass
class Sched:
    """Per-block slot counts (slots = 128-edge chunks), shared across cores."""
    slots: np.ndarray  # [BPC] int

    slot_block: list = field(default_factory=list)       # slot -> block
    group_call_slots: list = field(default_factory=list)  # [g] -> (s0, s1)
    block_slot_ranges: list = field(default_factory=list)  # [b] -> (s0, s1)
    total: int = 0

    def finalize(self, cfg: Cfg):
        self.slot_block = []
        self.group_call_slots = []
        self.block_slot_ranges = [None] * cfg.BPC
        s = 0
        for g in range(cfg.NGROUPS):
            s0g = s
            for b in cfg.group_blocks(g):
                bs0 = s
                for _ in range(int(self.slots[b])):
                    self.slot_block.append(b)
                    s += 1
                self.block_slot_ranges[b] = (bs0, s)
            self.group_call_slots.append((s0g, s))
        self.total = s


def gather_calls(cfg: Cfg, sched: "Sched"):
    """Static per-layer gather call ranges: [(s0, s1)] in slot space.
    One call per block (split by MAXC) so slot padding is always trailing
    within its call and can be skipped via negative idxs + num_idxs_reg."""
    calls = []
    for g in range(cfg.NGROUPS):
        for b in cfg.group_blocks(g):
            bs0, bs1 = sched.block_slot_ranges[b]
            for o0 in range(bs0, bs1, cfg.MAXC):
                calls.append((o0, min(o0 + cfg.MAXC, bs1), b))
    return calls


def make_schedule(edge_index: np.ndarray, cfg: Cfg):
    """Compute the shared slot schedule + per-core slot contents."""
    row = np.asarray(edge_index[0], dtype=np.int64)
    col = np.asarray(edge_index[1], dtype=np.int64)

    core = row // cfg.NPC
    rloc = row % cfg.NPC
    blk = rloc // cfg.BW
    rrel = (rloc % cfg.BW).astype(np.float32)
    # column ids live in the PADDED, AG-chunk-major table: chunk -> core ->
    # local pair row. With AGCHUNK=1 this is plain core-major.
    ccore = col // cfg.NPC
    clocal = col % cfg.NPC
    r = clocal >> 1                        # local pair row < NPAD/2
    par = (clocal & 1).astype(np.int64)    # parity within the pair
    chunks = cfg.ag_chunks()
    cbp = np.array([c[0] // 2 for c in chunks] + [cfg.NPAD // 2])  # pair bounds
    w = cbp[1:] - cbp[:-1]                 # chunk widths (pairs)
    off8 = np.concatenate([[0], np.cumsum(cfg.CORES * w)])
    ck = np.searchsorted(cbp, r, side="right") - 1
    pidx = (off8[ck] + ccore * w[ck] + (r - cbp[ck])).astype(np.int16)

    # group edges by (core, block); sort by pair id within each segment so
    # gather descriptors walk ascending HBM addresses
    key = core * cfg.BPC + blk
    order = np.lexsort((pidx, key))
    key_s = key[order]
    rrel_s = rrel[order]
    pidx_s = pidx[order]
    par_s = par[order]

    bounds = np.searchsorted(
        key_s, np.arange(cfg.CORES * cfg.BPC + 1), side="left"
    )
    counts = (bounds[1:] - bounds[:-1]).reshape(cfg.CORES, cfg.BPC)

    slots = np.max((counts + 127) // 128, axis=0)  # [BPC]

    sched = Sched(slots=slots)
    sched.finalize(cfg)

    TC = sched.total
    per_core = []
    NEG = np.float32(-100.0)
    for c in range(cfg.CORES):
        idx_flat = np.zeros((TC, 128), dtype=np.int16)
        rrE_flat = np.full((TC, 128), NEG, dtype=np.float32)
        rrO_flat = np.full((TC, 128), NEG, dtype=np.float32)
        for b in range(cfg.BPC):
            s0, s1 = sched.block_slot_ranges[b]
            cap = (s1 - s0) * 128
            k = c * cfg.BPC + b
            e0, e1 = bounds[k], bounds[k + 1]
            n = e1 - e0
            assert n <= cap, (c, b, n, cap)
            if cap == 0:
                continue
            ci = np.full(cap, -1, dtype=np.int16)
            rE = np.full(cap, NEG, dtype=np.float32)
            rO = np.full(cap, NEG, dtype=np.float32)
            ci[:n] = pidx_s[e0:e1]
            pe = par_s[e0:e1] == 0
            rE[:n] = np.where(pe, rrel_s[e0:e1], NEG)
            rO[:n] = np.where(~pe, rrel_s[e0:e1], NEG)
            idx_flat[s0:s1] = ci.reshape(-1, 128)
            rrE_flat[s0:s1] = rE.reshape(-1, 128)
            rrO_flat[s0:s1] = rO.reshape(-1, 128)

        # gather index tile layout: [128, TC*8] int16; logical edge k of
        # slot s lives at [k % 16, s*8 + k // 16], replicated across the
        # eight 16-partition groups.
        idx_tile = np.zeros((128, TC * 8), dtype=np.int16)
        base = idx_flat.reshape(TC, 8, 16).transpose(2, 0, 1).reshape(16, TC * 8)
        for rep in range(8):
            idx_tile[rep * 16:(rep + 1) * 16] = base

        # rr tiles: [128, TC]; partition = edge position in slot.
        rrE_tile = rrE_flat.T.astype(BF16)
        rrO_tile = rrO_flat.T.astype(BF16)

        # per-call valid (non-negative) idx counts for num_idxs_reg
        calls = gather_calls(cfg, sched)
        gcnt = np.zeros((1, len(calls)), dtype=np.uint32)
        for i, (o0, o1, b) in enumerate(calls):
            bs0, _bs1 = sched.block_slot_ranges[b]
            nvalid = int(counts[c, b])
            done = (o0 - bs0) * 128
            gcnt[0, i] = max(0, min(nvalid - done, (o1 - o0) * 128))

        per_core.append((idx_tile, rrE_tile, rrO_tile, gcnt))

    return sched, per_core


def host_inputs(x, edge_index, W, b, cfg: Cfg):
    """Build per-core in_maps (numpy only)."""
    x = np.asarray(x, dtype=np.float32)
    W = np.asarray(W, dtype=np.float32)
    b = np.asarray(b, dtype=np.float32)

    sched, per_core = make_schedule(edge_index, cfg)

    row = np.asarray(edge_index[0], dtype=np.int64)
    deg = np.bincount(row, minlength=cfg.N).astype(np.float32)
    norm = 1.0 / np.sqrt(1.0 + deg)

    iota64 = np.tile(np.arange(cfg.BW, dtype=np.float32), (128, 1)).astype(BF16)
    I64 = np.eye(cfg.BW, dtype=np.float32).astype(BF16)

    in_maps = []
    for c in range(cfg.CORES):
        r0 = c * cfg.NPC
        xT = np.zeros((cfg.IN, cfg.NPAD), dtype=np.float32)
        xT[:, : cfg.NPC] = x[r0 : r0 + cfg.NPC].T
        nc_ = np.ones(cfg.NPAD, dtype=np.float32)
        nc_[: cfg.NPC] = norm[r0 : r0 + cfg.NPC]
        norm64 = nc_.reshape(cfg.BPC, cfg.BW).T.copy()        # [64, BPC]
        norm264 = (norm64 * norm64).copy()
        idx_tile, rrE_tile, rrO_tile, gcnt = per_core[c]
        in_maps.append(
            {
                "xT": xT,
                "Wm": W.copy(),
                "bvals": b.reshape(1, cfg.HID).copy(),
                "iota64": iota64.copy(),
                "I64": I64.copy(),
                "norm64": norm64,
                "norm264": norm264,
                "idx_all": idx_tile,
                "gcnt": gcnt,
                "rrE_all": rrE_tile,
                "rrO_all": rrO_tile,
            }
        )
    return sched, in_maps


def build_bass(cfg: Cfg, sched: Sched, no_ag: bool = False, ablate=()):
    """Emit the Tile program. Returns compiled nc."""
    from concourse import bacc, bass, mybir, tile

    f32 = mybir.dt.float32
    bf16 = mybir.dt.bfloat16
    i16 = mybir.dt.int16
    EQ = mybir.AluOpType.is_equal
    COPY = mybir.ActivationFunctionType.Copy

    BW = cfg.BW
    TC = sched.total
    nc = bacc.Bacc(
        "TRN2",
        target_bir_lowering=False,
        debug=False,
        num_devices=cfg.CORES,
        num_swdge_queues=4,
        dynamic_dma_scratch_size=cfg.SCRATCH,
    )

    # ---- I/O ----
    xT_d = nc.dram_tensor("xT", [cfg.IN, cfg.NPAD], f32, kind="ExternalInput")
    W_d = nc.dram_tensor("Wm", [cfg.IN, cfg.HID], f32, kind="ExternalInput")
    b_d = nc.dram_tensor("bvals", [1, cfg.HID], f32, kind="ExternalInput")
    iota_d = nc.dram_tensor("iota64", [128, BW], bf16, kind="ExternalInput")
    I64_d = nc.dram_tensor("I64", [BW, BW], bf16, kind="ExternalInput")
    n1_d = nc.dram_tensor("norm64", [BW, cfg.BPC], f32, kind="ExternalInput")
    n2_d = nc.dram_tensor("norm264", [BW, cfg.BPC], f32, kind="ExternalInput")
    idx_d = nc.dram_tensor("idx_all", [128, TC * 8], i16, kind="ExternalInput")
    _calls = gather_calls(cfg, sched)
    gcnt_d = nc.dram_tensor(
        "gcnt", [1, len(_calls)], mybir.dt.uint32, kind="ExternalInput"
    )
    rrE_d = nc.dram_tensor("rrE_all", [128, TC], bf16, kind="ExternalInput")
    rrO_d = nc.dram_tensor("rrO_all", [128, TC], bf16, kind="ExternalInput")
    out_d = nc.dram_tensor("out", [cfg.NPAD, cfg.HID], f32, kind="ExternalOutput")

    # internal DRAM: AG input (own xs rows) + AG output (paired full table)
    xs_in = [
        nc.dram_tensor(f"xs_in{l}", [cfg.NPAD, cfg.HID], bf16)
        for l in range(cfg.LAYERS)
    ]
    _aspace = "Shared" if cfg.CORES > 4 else "Local"
    NPAIR = cfg.CORES * cfg.NPAD // 2  # 25088 paired rows of 128 bf16
    xs_full = [
        nc.dram_tensor(
            f"xs_full{l}", [NPAIR, 2 * cfg.HID], bf16, addr_space=_aspace
        )
        for l in range(cfg.LAYERS)
    ]

    rg = [list(range(cfg.CORES))]
    cnt_regs = [nc.gpsimd.alloc_register(f"gcnt_reg{i}") for i in range(4)]

    with tile.TileContext(nc) as tc:
        with (
            tc.tile_pool(name="const", bufs=1) as constp,
            tc.tile_pool(name="gbuf", bufs=cfg.GBUFS) as gpool,
            tc.tile_pool(name="onehot", bufs=cfg.OBUFS) as opool,
            tc.tile_pool(name="xtp", bufs=2) as xtp,
            tc.tile_pool(name="res", bufs=2) as resp,
            tc.tile_pool(name="psum", bufs=8, space="PSUM") as psp,
        ):
            # ---- persistent SBUF ----
            W_s = constp.tile([cfg.IN, cfg.HID], f32, tag="W")
            b_s = constp.tile([1, cfg.HID], f32, tag="b")
            ones_s = constp.tile([1, cfg.HID], f32, tag="ones")
            iota_s = constp.tile([128, BW], bf16, tag="iota")
            I64_s = constp.tile([BW, BW], bf16, tag="I64")
            n1_s = constp.tile([BW, cfg.BPC], f32, tag="n1")
            n2_s = constp.tile([BW, cfg.BPC], f32, tag="n2")
            idx_s = constp.tile([128, TC * 8], i16, tag="idx")
            gcnt_s = constp.tile([1, len(_calls)], mybir.dt.uint32, tag="gcnt")
            rrE_s = constp.tile([128, TC], bf16, tag="rrE")
            rrO_s = constp.tile([128, TC], bf16, tag="rrO")
            xs_ping = constp.tile([BW, cfg.BPC, cfg.HID], bf16, tag="xsA")
            xs_pong = constp.tile([BW, cfg.BPC, cfg.HID], bf16, tag="xsB")

            nc.sync.dma_start(W_s[:], W_d[:, :])
            nc.sync.dma_start(b_s[:], b_d[:, :])
            nc.sync.dma_start(iota_s[:], iota_d[:, :])
            nc.sync.dma_start(I64_s[:], I64_d[:, :])
            nc.sync.dma_start(n1_s[:], n1_d[:, :])
            nc.sync.dma_start(n2_s[:], n2_d[:, :])
            nc.sync.dma_start(idx_s[:], idx_d[:, :])
            nc.sync.dma_start(gcnt_s[:], gcnt_d[:, :])
            nc.sync.dma_start(rrE_s[:], rrE_d[:, :])
            nc.sync.dma_start(rrO_s[:], rrO_d[:, :])
            nc.vector.memset(ones_s[:], 1.0)
            # pre-touch every gather buffer: calls skip trailing padding, so
            # padded G rows are never written and must start finite (the
            # one-hot zeroes their contribution, but 0 * garbage-NaN = NaN).
            _maxnch = max(
                (sched.group_call_slots[g][1] - sched.group_call_slots[g][0])
                for g in range(cfg.NGROUPS)
            )
            for _i in range(cfg.GBUFS):
                Ginit = gpool.tile([128, _maxnch, 2 * cfg.HID], bf16, tag="G")
                nc.vector.memset(Ginit[:], 0.0)

            def store_group_to_dram(dram, g, src_ap):
                """src_ap [BW, nb, HID] -> dram rows [BW*b0, BW*b1)."""
                blocks = cfg.group_blocks(g)
                b0, b1 = blocks[0], blocks[-1] + 1
                dst = dram[BW * b0 : BW * b1, :].rearrange(
                    "(b p) h -> p b h", p=BW
                )
                nc.sync.dma_start(dst, src_ap)

            # ---- prologue: h0 = x @ W + b ; xs0 = norm * h0 ----
            for g in range(cfg.NGROUPS):
                blocks = cfg.group_blocks(g)
                nb = len(blocks)
                b0 = blocks[0]
                xtg = xtp.tile([cfg.IN, nb * BW], f32, tag="xtg")
                nc.sync.dma_start(
                    xtg[:], xT_d[:, BW * b0 : BW * (b0 + nb)]
                )
                for j, b in enumerate(blocks):
                    ps = psp.tile([BW, cfg.HID], f32, tag="ps")
                    nc.tensor.matmul(
                        ps[:], ones_s[:], b_s[:], start=True, stop=False
                    )
                    nc.tensor.matmul(
                        ps[:],
                        xtg[:, BW * j : BW * (j + 1)],
                        W_s[:],
                        start=False,
                        stop=True,
                    )
                    nc.scalar.activation(
                        xs_ping[:, b, :], ps[:], COPY,
                        scale=n1_s[:, b : b + 1],
                    )
                store_group_to_dram(xs_in[0], g, xs_ping[:, b0 : b0 + nb, :])

            # AG chunk boundaries: contiguous group ranges, each ending at a
            # group boundary so a chunk can launch as soon as its groups store.
            _chunks = cfg.ag_chunks()
            _chunk_end_group = {eg: i for i, (_r0, _r1, eg) in enumerate(_chunks)}
            _off8 = np.concatenate(
                [[0], np.cumsum([cfg.CORES * (r1 - r0) // 2 for r0, r1, _ in _chunks])]
            ).astype(int)

            def do_ag(l, k=None):
                if "tinyag" in ablate:
                    if k not in (None, 0):
                        return
                    nc.gpsimd.collective_compute(
                        "AllGather", mybir.AluOpType.bypass, replica_groups=rg,
                        ins=[xs_in[l][0:1, 0:8]], outs=[xs_full[l][0:1, 0:64]],
                    )
                    return
                if k is None:
                    ks = range(len(_chunks))
                else:
                    ks = [k]
                for kk in ks:
                    r0, r1, _eg = _chunks[kk]
                    o0 = _off8[kk]
                    o1 = o0 + cfg.CORES * (r1 - r0) // 2
                    nc.gpsimd.collective_compute(
                        "AllGather", mybir.AluOpType.bypass, replica_groups=rg,
                        ins=[xs_in[l][r0:r1, :]],
                        outs=[xs_full[l][o0:o1, :]],
                    )

            if not no_ag:
                do_ag(0)

            # ---- layers ----
            xs_cur, xs_nxt = xs_ping, xs_pong
            for _rep in range(cfg.REPEAT):
              for l in range(cfg.LAYERS):
                last = l == cfg.LAYERS - 1
                table = xs_full[l]
                qn = 0
                for g in range(cfg.NGROUPS):
                    blocks = cfg.group_blocks(g)
                    nb = len(blocks)
                    b0 = blocks[0]
                    s0, s1 = sched.group_call_slots[g]
                    nch = s1 - s0
                    G = None
                    ohE = ohO = None
                    if nch > 0:
                        G = gpool.tile([128, nch, 2 * cfg.HID], bf16, tag="G")
                        if "gather" in ablate:
                            nc.gpsimd.dma_gather(
                                G[:, 0:1, :], table[:, :], idx_s[:, 8 * s0 : 8 * (s0 + 1)],
                                128, 128, 2 * cfg.HID,
                                single_packet=cfg.SP, queue_num=qn % 4,
                            )
                            qn += 1
                        else:
                            for ci_, (o0, o1, _b) in enumerate(_calls):
                                if o0 < s0 or o0 >= s1:
                                    continue
                                n = o1 - o0
                                cnt = cnt_regs[qn % 4]
                                nc.gpsimd.reg_load(
                                    cnt, gcnt_s[0:1, ci_ : ci_ + 1]
                                )
                                nc.gpsimd.dma_gather(
                                    G[:, o0 - s0 : o1 - s0, :],
                                    table[:, :],
                                    idx_s[:, 8 * o0 : 8 * o1],
                                    n * 128,
                                    cnt,
                                    2 * cfg.HID,
                                    single_packet=cfg.SP,
                                    queue_num=qn % 4,
                                )
                                qn += 1
                        ohE = opool.tile([128, nch, BW], bf16, tag="ohE")
                        ohO = opool.tile([128, nch, BW], bf16, tag="ohO")
                        if "oh" in ablate:
                            nc.vector.memset(ohE[:], 0.0)
                            nc.vector.memset(ohO[:], 0.0)
                        else:
                            io_b = (
                                iota_s[:, :]
                                .unsqueeze(1)
                                .broadcast_to([128, nch, BW])
                            )
                            rE_b = (
                                rrE_s[:, s0:s1]
                                .unsqueeze(2)
                                .broadcast_to([128, nch, BW])
                            )
                            rO_b = (
                                rrO_s[:, s0:s1]
                                .unsqueeze(2)
                                .broadcast_to([128, nch, BW])
                            )
                            nc.vector.tensor_tensor(ohE[:], io_b, rE_b, EQ)
                            nc.vector.tensor_tensor(ohO[:], io_b, rO_b, EQ)

                    rtile = None
                    if last:
                        rtile = resp.tile([BW, nb, cfg.HID], f32, tag="r")
                    for j, b in enumerate(blocks):
                        ps = psp.tile([BW, cfg.HID], f32, tag="ps")
                        bs0, bs1 = sched.block_slot_ranges[b]
                        k = 0
                        if "mm" not in ablate:
                            for s in range(bs0 - s0, bs1 - s0):
                                nc.tensor.matmul(
                                    ps[:],
                                    ohE[:, s, :],
                                    G[:, s, 0 : cfg.HID],
                                    start=(k == 0),
                                    stop=False,
                                )
                                k += 1
                                nc.tensor.matmul(
                                    ps[:],
                                    ohO[:, s, :],
                                    G[:, s, cfg.HID : 2 * cfg.HID],
                                    start=False,
                                    stop=False,
                                )
                                k += 1
                        # self-term: ps += I @ xs_cur[b]
                        nc.tensor.matmul(
                            ps[:],
                            I64_s[:],
                            xs_cur[:, b, :],
                            start=(k == 0),
                            stop=True,
                        )
                        nsrc = n1_s if last else n2_s
                        dst = rtile[:, j, :] if last else xs_nxt[:, b, :]
                        nc.scalar.activation(
                            dst, ps[:], COPY, scale=nsrc[:, b : b + 1]
                        )
                    if last:
                        store_group_to_dram(out_d, g, rtile[:])
                    else:
                        store_group_to_dram(
                            xs_in[l + 1], g, xs_nxt[:, b0 : b0 + nb, :]
                        )
                        if not no_ag and g in _chunk_end_group:
                            do_ag(l + 1, _chunk_end_group[g])

                if not last:
                    xs_cur, xs_nxt = xs_nxt, xs_cur

    nc.compile()
    return nc


def bench_bass(nc, in_maps, n_cores, iters=20, warmup=2):
    """Repeat-execution device benchmark (no NTFF tracing in this container).

    Mirrors bass2jax.run_bass_via_pjrt's multi-core path, minus output-buffer
    donation so the compiled executable can be re-invoked. Returns
    (results_list, per_iter_seconds).
    """
    import time

    import jax
    from jax.experimental.shard_map import shard_map
    from jax.sharding import Mesh, NamedSharding, PartitionSpec

    from concourse import bass2jax, mybir

    bass2jax.install_neuronx_cc_hook()

    partition_name = (
        nc.partition_id_tensor.name if nc.partition_id_tensor else None
    )
    in_names, out_names, out_avals, zero_outs = [], [], [], []
    for alloc in nc.m.functions[0].allocations:
        if not isinstance(alloc, mybir.MemoryLocationSet):
            continue
        name = alloc.memorylocations[0].name
        if alloc.kind == "ExternalInput":
            if name != partition_name:
                in_names.append(name)
        elif alloc.kind == "ExternalOutput":
            out_names.append(name)
            shape = tuple(alloc.tensor_shape)
            dtype = mybir.dt.np(alloc.dtype)
            out_avals.append(jax.core.ShapedArray(shape, dtype))
            zero_outs.append(np.zeros(shape, dtype))
    n_params = len(in_names)
    all_names = in_names + out_names
    if partition_name is not None:
        all_names = all_names + [partition_name]

    def _body(*args):
        operands = list(args)
        if partition_name is not None:
            operands.append(bass2jax.partition_id_tensor())
        outs = bass2jax._bass_exec_p.bind(
            *operands,
            out_avals=tuple(out_avals),
            in_names=tuple(all_names),
            out_names=tuple(out_names),
            lowering_input_output_aliases=(),
            sim_require_finite=True,
            sim_require_nnan=True,
            nc=nc,
        )
        return tuple(outs)

    devices = jax.devices()[:n_cores]
    mesh = Mesh(np.asarray(devices), ("core",))
    spec = PartitionSpec("core")
    nin = n_params + len(zero_outs)
    sharded = jax.jit(
        shard_map(
            _body,
            mesh=mesh,
            in_specs=(spec,) * nin,
            out_specs=(spec,) * len(out_names),
            check_rep=False,
        ),
        keep_unused=True,
    )
    sh = NamedSharding(mesh, spec)
    args = [
        jax.device_put(
            np.concatenate([np.asarray(m[name]) for m in in_maps], axis=0), sh
        )
        for name in in_names
    ] + [
        jax.device_put(
            np.zeros((n_cores * z.shape[0], *z.shape[1:]), z.dtype), sh
        )
        for z in zero_outs
    ]

    out_arrs = None
    for _ in range(warmup):
        out_arrs = sharded(*args)
        jax.block_until_ready(out_arrs)
    t0 = time.perf_counter()
    for _ in range(iters):
        out_arrs = sharded(*args)
    jax.block_until_ready(out_arrs)
    t1 = time.perf_counter()

    results = [
        {
            name: np.asarray(out_arrs[i]).reshape(n_cores, *out_avals[i].shape)[c]
            for i, name in enumerate(out_names)
        }
        for c in range(n_cores)
    ]
    return results, (t1 - t0) / iters


def kernel(x, edge_index, W, b, cfg: Cfg | None = None, trace: bool = False):
    """Full-input entry point. Returns [N, HID] float32 (+ results if trace)."""
    cfg = cfg or Cfg()
    sched, in_maps = host_inputs(x, edge_index, W, b, cfg)
    nc = build_bass(cfg, sched)

    from concourse import bass_utils

    res = bass_utils.run_bass_kernel_spmd(
        nc,
        in_maps,
        core_ids=list(range(cfg.CORES)),
        trace=False,
    )
    out = np.concatenate(
        [r["out"][: cfg.NPC] for r in res.results], axis=0
    )
    if trace:
        return out, res
    return out


if __name__ == "__main__":
    pass


# revision 14
# speedup vs baseline: 1.0617x; 1.0102x over previous
"""GCN (3-layer message passing) Trainium2 Bass kernel, 8-way node-sharded.

v2 strategy (vs v1: fp32 single-row gather, 128-wide one-hots, DVE scaling):
  - Feature table in bf16. dma_gather requires 256B-multiple elements, so
    rows are gathered in PAIRS (256B = 2 bf16 rows, index = padded_row >> 1).
    Pair indices span [0, 25088) < 2^15, so one int16 index space (no table
    halves).
  - Dest rows blocked 64-wide: halves the one-hot build work on DVE (the
    broadcast is_equal build gets no 16-bit speedup, so width is the lever).
  - Per 128-edge slot, TWO bf16 one-hot matmuls (even/odd pair parity pick
    the correct 64-col half of the gathered pair) at 1 cycle/row.
  - The "+ xs" self-term folds into PSUM via an identity matmul; the final
    norm/norm^2 scaling runs on the otherwise-idle Activation engine
    (Copy with per-partition scale), leaving DVE with only one-hot builds.
  - Per layer: AllGather bf16 table (half of v1 traffic) -> per-edge pair
    gather -> one-hot matmuls into [64,64] PSUM accumulators -> ACT scale.

Self-contained: hardcodes the problem shapes; only needs numpy + the
concourse stack at /opt/trn_rl_repo.
"""

import sys

for _p in ("/opt/trn_rl_repo",):
    if _p not in sys.path:
        sys.path.insert(0, _p)

from dataclasses import dataclass, field

import numpy as np
import ml_dtypes

BF16 = ml_dtypes.bfloat16


@dataclass
class Cfg:
    N: int = 50000
    IN: int = 128
    HID: int = 64
    LAYERS: int = 3
    CORES: int = 8
    BW: int = 64       # dest block width (one-hot free dim)
    GB: int = 8        # 64-blocks per processing group
    MAXC: int = 12     # max chunks (x128 idxs) per dma_gather call
    GBUFS: int = 3     # gather-tile double/triple buffering
    OBUFS: int = 3     # one-hot tile buffering
    SCRATCH: int = 16384  # dynamic dma scratch (swdge ring) bytes/partition
    SP: bool = False   # dma_gather single_packet mode
    AGCHUNK: int = 4   # split each AllGather into this many row-chunks
    REPEAT: int = 1    # repeat the layer stack (timing experiments only)

    @property
    def NPC(self):  # nodes per core
        assert self.N % self.CORES == 0
        return self.N // self.CORES

    @property
    def BPC(self):  # BW-row blocks per core
        return (self.NPC + self.BW - 1) // self.BW

    @property
    def NPAD(self):
        return self.BPC * self.BW

    @property
    def NGROUPS(self):
        return (self.BPC + self.GB - 1) // self.GB

    def group_blocks(self, g):
        return list(range(g * self.GB, min((g + 1) * self.GB, self.BPC)))

    def ag_chunks(self):
        """[(row0, row1, end_group)] per AG chunk; chunk-major table layout."""
        splits = [a for a in np.array_split(np.arange(self.NGROUPS), self.AGCHUNK) if len(a)]
        out = []
        for a in splits:
            r0 = int(a[0]) * self.GB * self.BW
            r1 = min((int(a[-1]) + 1) * self.GB * self.BW, self.NPAD)
            out.append((r0, r1, int(a[-1])))
        return out


@dataclass
class Sched:
    """Per-block slot counts (slots = 128-edge chunks), shared across cores."""
    slots: np.ndarray  # [BPC] int

    slot_block: list = field(default_factory=list)       # slot -> block
    group_call_slots: list = field(default_factory=list)  # [g] -> (s0, s1)
    block_slot_ranges: list = field(default_factory=list)  # [b] -> (s0, s1)
    total: int = 0

    def finalize(self, cfg: Cfg):
        self.slot_block = []
        self.group_call_slots = []
        self.block_slot_ranges = [None] * cfg.BPC
        s = 0
        for g in range(cfg.NGROUPS):
            s0g = s
            for b in cfg.group_blocks(g):
                bs0 = s
                for _ in range(int(self.slots[b])):
                    self.slot_block.append(b)
                    s += 1
                self.block_slot_ranges[b] = (bs0, s)
            self.group_call_slots.append((s0g, s))
        self.total = s


def gather_calls(cfg: Cfg, sched: "Sched"):
    """Static per-layer gather call ranges: [(s0, s1)] in slot space.
    One call per block (split by MAXC) so slot padding is always trailing
    within its call and can be skipped via negative idxs + num_idxs_reg."""
    calls = []
    for g in range(cfg.NGROUPS):
        for b in cfg.group_blocks(g):
            bs0, bs1 = sched.block_slot_ranges[b]
            for o0 in range(bs0, bs1, cfg.MAXC):
                calls.append((o0, min(o0 + cfg.MAXC, bs1), b))
    return calls


def make_schedule(edge_index: np.ndarray, cfg: Cfg):
    """Compute the shared slot schedule + per-core slot contents."""
    row = np.asarray(edge_index[0], dtype=np.int64)
    col = np.asarray(edge_index[1], dtype=np.int64)

    core = row // cfg.NPC
    rloc = row % cfg.NPC
    blk = rloc // cfg.BW
    rrel = (rloc % cfg.BW).astype(np.float32)
    # column ids live in the PADDED, AG-chunk-major table: chunk -> core ->
    # local pair row. With AGCHUNK=1 this is plain core-major.
    ccore = col // cfg.NPC
    clocal = col % cfg.NPC
    r = clocal >> 1                        # local pair row < NPAD/2
    par = (clocal & 1).astype(np.int64)    # parity within the pair
    chunks = cfg.ag_chunks()
    cbp = np.array([c[0] // 2 for c in chunks] + [cfg.NPAD // 2])  # pair bounds
    w = cbp[1:] - cbp[:-1]                 # chunk widths (pairs)
    off8 = np.concatenate([[0], np.cumsum(cfg.CORES * w)])
    ck = np.searchsorted(cbp, r, side="right") - 1
    pidx = (off8[ck] + ccore * w[ck] + (r - cbp[ck])).astype(np.int16)

    # group edges by (core, block); sort by pair id within each segment so
    # gather descriptors walk ascending HBM addresses
    key = core * cfg.BPC + blk
    order = np.lexsort((pidx, key))
    key_s = key[order]
    rrel_s = rrel[order]
    pidx_s = pidx[order]
    par_s = par[order]

    bounds = np.searchsorted(
        key_s, np.arange(cfg.CORES * cfg.BPC + 1), side="left"
    )
    counts = (bounds[1:] - bounds[:-1]).reshape(cfg.CORES, cfg.BPC)

    slots = np.max((counts + 127) // 128, axis=0)  # [BPC]

    sched = Sched(slots=slots)
    sched.finalize(cfg)

    TC = sched.total
    per_core = []
    NEG = np.float32(-100.0)
    for c in range(cfg.CORES):
        idx_flat = np.zeros((TC, 128), dtype=np.int16)
        rrE_flat = np.full((TC, 128), NEG, dtype=np.float32)
        rrO_flat = np.full((TC, 128), NEG, dtype=np.float32)
        for b in range(cfg.BPC):
            s0, s1 = sched.block_slot_ranges[b]
            cap = (s1 - s0) * 128
            k = c * cfg.BPC + b
            e0, e1 = bounds[k], bounds[k + 1]
            n = e1 - e0
            assert n <= cap, (c, b, n, cap)
            if cap == 0:
                continue
            ci = np.full(cap, -1, dtype=np.int16)
            rE = np.full(cap, NEG, dtype=np.float32)
            rO = np.full(cap, NEG, dtype=np.float32)
            ci[:n] = pidx_s[e0:e1]
            pe = par_s[e0:e1] == 0
            rE[:n] = np.where(pe, rrel_s[e0:e1], NEG)
            rO[:n] = np.where(~pe, rrel_s[e0:e1], NEG)
            idx_flat[s0:s1] = ci.reshape(-1, 128)
            rrE_flat[s0:s1] = rE.reshape(-1, 128)
            rrO_flat[s0:s1] = rO.reshape(-1, 128)

        # gather index tile layout: [128, TC*8] int16; logical edge k of
        # slot s lives at [k % 16, s*8 + k // 16], replicated across the
        # eight 16-partition groups.
        idx_tile = np.zeros((128, TC * 8), dtype=np.int16)
        base = idx_flat.reshape(TC, 8, 16).transpose(2, 0, 1).reshape(16, TC * 8)
        for rep in range(8):
            idx_tile[rep * 16:(rep + 1) * 16] = base

        # rr tiles: [128, TC]; partition = edge position in slot.
        rrE_tile = rrE_flat.T.astype(BF16)
        rrO_tile = rrO_flat.T.astype(BF16)

        # per-call valid (non-negative) idx counts for num_idxs_reg
        calls = gather_calls(cfg, sched)
        gcnt = np.zeros((1, len(calls)), dtype=np.uint32)
        for i, (o0, o1, b) in enumerate(calls):
            bs0, _bs1 = sched.block_slot_ranges[b]
            nvalid = int(counts[c, b])
            done = (o0 - bs0) * 128
            gcnt[0, i] = max(0, min(nvalid - done, (o1 - o0) * 128))

        per_core.append((idx_tile, rrE_tile, rrO_tile, gcnt))

    return sched, per_core


def host_inputs(x, edge_index, W, b, cfg: Cfg):
    """Build per-core in_maps (numpy only)."""
    x = np.asarray(x, dtype=np.float32)
    W = np.asarray(W, dtype=np.float32)
    b = np.asarray(b, dtype=np.float32)

    sched, per_core = make_schedule(edge_index, cfg)

    row = np.asarray(edge_index[0], dtype=np.int64)
    deg = np.bincount(row, minlength=cfg.N).astype(np.float32)
    norm = 1.0 / np.sqrt(1.0 + deg)

    iota64 = np.tile(np.arange(cfg.BW, dtype=np.float32), (128, 1)).astype(BF16)
    I64 = np.eye(cfg.BW, dtype=np.float32).astype(BF16)

    in_maps = []
    for c in range(cfg.CORES):
        r0 = c * cfg.NPC
        xT = np.zeros((cfg.IN, cfg.NPAD), dtype=np.float32)
        xT[:, : cfg.NPC] = x[r0 : r0 + cfg.NPC].T
        nc_ = np.ones(cfg.NPAD, dtype=np.float32)
        nc_[: cfg.NPC] = norm[r0 : r0 + cfg.NPC]
        norm64 = nc_.reshape(cfg.BPC, cfg.BW).T.copy()        # [64, BPC]
        norm264 = (norm64 * norm64).copy()
        idx_tile, rrE_tile, rrO_tile, gcnt = per_core[c]
        in_maps.append(
            {
                "xT": xT,
                "Wm": W.copy(),
                "bvals": b.reshape(1, cfg.HID).copy(),
                "iota64": iota64.copy(),
                "I64": I64.copy(),
                "norm64": norm64,
                "norm264": norm264,
                "idx_all": idx_tile,
                "gcnt": gcnt,
                "rrE_all": rrE_tile,
                "rrO_all": rrO_tile,
            }
        )
    return sched, in_maps


def build_bass(cfg: Cfg, sched: Sched, no_ag: bool = False, ablate=()):
    """Emit the Tile program. Returns compiled nc."""
    from concourse import bacc, bass, mybir, tile

    f32 = mybir.dt.float32
    bf16 = mybir.dt.bfloat16
    i16 = mybir.dt.int16
    EQ = mybir.AluOpType.is_equal
    COPY = mybir.ActivationFunctionType.Copy

    BW = cfg.BW
    TC = sched.total
    nc = bacc.Bacc(
        "TRN2",
        target_bir_lowering=False,
        debug=False,
        num_devices=cfg.CORES,
        num_swdge_queues=4,
        dynamic_dma_scratch_size=cfg.SCRATCH,
    )

    # ---- I/O ----
    xT_d = nc.dram_tensor("xT", [cfg.IN, cfg.NPAD], f32, kind="ExternalInput")
    W_d = nc.dram_tensor("Wm", [cfg.IN, cfg.HID], f32, kind="ExternalInput")
    b_d = nc.dram_tensor("bvals", [1, cfg.HID], f32, kind="ExternalInput")
    iota_d = nc.dram_tensor("iota64", [128, BW], bf16, kind="ExternalInput")
    I64_d = nc.dram_tensor("I64", [BW, BW], bf16, kind="ExternalInput")
    n1_d = nc.dram_tensor("norm64", [BW, cfg.BPC], f32, kind="ExternalInput")
    n2_d = nc.dram_tensor("norm264", [BW, cfg.BPC], f32, kind="ExternalInput")
    idx_d = nc.dram_tensor("idx_all", [128, TC * 8], i16, kind="ExternalInput")
    _calls = gather_calls(cfg, sched)
    gcnt_d = nc.dram_tensor(
        "gcnt", [1, len(_calls)], mybir.dt.uint32, kind="ExternalInput"
    )
    rrE_d = nc.dram_tensor("rrE_all", [128, TC], bf16, kind="ExternalInput")
    rrO_d = nc.dram_tensor("rrO_all", [128, TC], bf16, kind="ExternalInput")
    out_d = nc.dram_tensor("out", [cfg.NPAD, cfg.HID], f32, kind="ExternalOutput")

    # internal DRAM: AG input (own xs rows) + AG output (paired full table)
    xs_in = [
        nc.dram_tensor(f"xs_in{l}", [cfg.NPAD, cfg.HID], bf16)
        for l in range(cfg.LAYERS)
    ]
    _aspace = "Shared" if cfg.CORES > 4 else "Local"
    NPAIR = cfg.CORES * cfg.NPAD // 2  # 25088 paired rows of 128 bf16
    xs_full = [
        nc.dram_tensor(
            f"xs_full{l}", [NPAIR, 2 * cfg.HID], bf16, addr_space=_aspace
        )
        for l in range(cfg.LAYERS)
    ]

    rg = [list(range(cfg.CORES))]
    cnt_regs = [nc.gpsimd.alloc_register(f"gcnt_reg{i}") for i in range(4)]

    with tile.TileContext(nc) as tc:
        with (
            tc.tile_pool(name="const", bufs=1) as constp,
            tc.tile_pool(name="gbuf", bufs=cfg.GBUFS) as gpool,
            tc.tile_pool(name="onehot", bufs=cfg.OBUFS) as opool,
            tc.tile_pool(name="xtp", bufs=2) as xtp,
            tc.tile_pool(name="res", bufs=2) as resp,
            tc.tile_pool(name="psum", bufs=8, space="PSUM") as psp,
        ):
            # ---- persistent SBUF ----
            W_s = constp.tile([cfg.IN, cfg.HID], f32, tag="W")
            b_s = constp.tile([1, cfg.HID], f32, tag="b")
            ones_s = constp.tile([1, cfg.HID], f32, tag="ones")
            iota_s = constp.tile([128, BW], bf16, tag="iota")
            I64_s = constp.tile([BW, BW], bf16, tag="I64")
            n1_s = constp.tile([BW, cfg.BPC], f32, tag="n1")
            n2_s = constp.tile([BW, cfg.BPC], f32, tag="n2")
            idx_s = constp.tile([128, TC * 8], i16, tag="idx")
            gcnt_s = constp.tile([1, len(_calls)], mybir.dt.uint32, tag="gcnt")
            rrE_s = constp.tile([128, TC], bf16, tag="rrE")
            rrO_s = constp.tile([128, TC], bf16, tag="rrO")
            xs_ping = constp.tile([BW, cfg.BPC, cfg.HID], bf16, tag="xsA")
            xs_pong = constp.tile([BW, cfg.BPC, cfg.HID], bf16, tag="xsB")

            nc.sync.dma_start(W_s[:], W_d[:, :])
            nc.sync.dma_start(b_s[:], b_d[:, :])
            nc.sync.dma_start(iota_s[:], iota_d[:, :])
            nc.sync.dma_start(I64_s[:], I64_d[:, :])
            nc.sync.dma_start(n1_s[:], n1_d[:, :])
            nc.sync.dma_start(n2_s[:], n2_d[:, :])
            nc.sync.dma_start(idx_s[:], idx_d[:, :])
            nc.sync.dma_start(gcnt_s[:], gcnt_d[:, :])
            nc.sync.dma_start(rrE_s[:], rrE_d[:, :])
            nc.sync.dma_start(rrO_s[:], rrO_d[:, :])
            nc.vector.memset(ones_s[:], 1.0)
            # pre-touch every gather buffer: calls skip trailing padding, so
            # padded G rows are never written and must start finite (the
            # one-hot zeroes their contribution, but 0 * garbage-NaN = NaN).
            _maxnch = max(
                (sched.group_call_slots[g][1] - sched.group_call_slots[g][0])
                for g in range(cfg.NGROUPS)
            )
            for _i in range(cfg.GBUFS):
                Ginit = gpool.tile([128, _maxnch, 2 * cfg.HID], bf16, tag="G")
                nc.vector.memset(Ginit[:], 0.0)

            def store_group_to_dram(dram, g, src_ap):
                """src_ap [BW, nb, HID] -> dram rows [BW*b0, BW*b1)."""
                blocks = cfg.group_blocks(g)
                b0, b1 = blocks[0], blocks[-1] + 1
                dst = dram[BW * b0 : BW * b1, :].rearrange(
                    "(b p) h -> p b h", p=BW
                )
                nc.sync.dma_start(dst, src_ap)

            # ---- prologue: h0 = x @ W + b ; xs0 = norm * h0 ----
            for g in range(cfg.NGROUPS):
                blocks = cfg.group_blocks(g)
                nb = len(blocks)
                b0 = blocks[0]
                xtg = xtp.tile([cfg.IN, nb * BW], f32, tag="xtg")
                nc.sync.dma_start(
                    xtg[:], xT_d[:, BW * b0 : BW * (b0 + nb)]
                )
                for j, b in enumerate(blocks):
                    ps = psp.tile([BW, cfg.HID], f32, tag="ps")
                    nc.tensor.matmul(
                        ps[:], ones_s[:], b_s[:], start=True, stop=False
                    )
                    nc.tensor.matmul(
                        ps[:],
                        xtg[:, BW * j : BW * (j + 1)],
                        W_s[:],
                        start=False,
                        stop=True,
                    )
                    nc.scalar.activation(
                        xs_ping[:, b, :], ps[:], COPY,
                        scale=n1_s[:, b : b + 1],
                    )
                store_group_to_dram(xs_in[0], g, xs_ping[:, b0 : b0 + nb, :])

            # AG chunk boundaries: contiguous group ranges, each ending at a
            # group boundary so a chunk can launch as soon as its groups store.
            _chunks = cfg.ag_chunks()
            _chunk_end_group = {eg: i for i, (_r0, _r1, eg) in enumerate(_chunks)}
            _off8 = np.concatenate(
                [[0], np.cumsum([cfg.CORES * (r1 - r0) // 2 for r0, r1, _ in _chunks])]
            ).astype(int)

            def do_ag(l, k=None):
                if "tinyag" in ablate:
                    if k not in (None, 0):
                        return
                    nc.gpsimd.collective_compute(
                        "AllGather", mybir.AluOpType.bypass, replica_groups=rg,
                        ins=[xs_in[l][0:1, 0:8]], outs=[xs_full[l][0:1, 0:64]],
                    )
                    return
                if k is None:
                    ks = range(len(_chunks))
                else:
                    ks = [k]
                for kk in ks:
                    r0, r1, _eg = _chunks[kk]
                    o0 = _off8[kk]
                    o1 = o0 + cfg.CORES * (r1 - r0) // 2
                    nc.gpsimd.collective_compute(
                        "AllGather", mybir.AluOpType.bypass, replica_groups=rg,
                        ins=[xs_in[l][r0:r1, :]],
                        outs=[xs_full[l][o0:o1, :]],
                    )

            if not no_ag:
                do_ag(0)

            # ---- layers ----
            xs_cur, xs_nxt = xs_ping, xs_pong
            for _rep in range(cfg.REPEAT):
              for l in range(cfg.LAYERS):
                last = l == cfg.LAYERS - 1
                table = xs_full[l]
                qn = 0
                for g in range(cfg.NGROUPS):
                    blocks = cfg.group_blocks(g)
                    nb = len(blocks)
                    b0 = blocks[0]
                    s0, s1 = sched.group_call_slots[g]
                    nch = s1 - s0
                    G = None
                    ohE = ohO = None
                    if nch > 0:
                        G = gpool.tile([128, nch, 2 * cfg.HID], bf16, tag="G")
                        if "gather" in ablate:
                            nc.gpsimd.dma_gather(
                                G[:, 0:1, :], table[:, :], idx_s[:, 8 * s0 : 8 * (s0 + 1)],
                                128, 128, 2 * cfg.HID,
                                single_packet=cfg.SP, queue_num=qn % 4,
                            )
                            qn += 1
                        else:
                            for ci_, (o0, o1, _b) in enumerate(_calls):
                                if o0 < s0 or o0 >= s1:
                                    continue
                                n = o1 - o0
                                cnt = cnt_regs[qn % 4]
                                nc.gpsimd.reg_load(
                                    cnt, gcnt_s[0:1, ci_ : ci_ + 1]
                                )
                                nc.gpsimd.dma_gather(
                                    G[:, o0 - s0 : o1 - s0, :],
                                    table[:, :],
                                    idx_s[:, 8 * o0 : 8 * o1],
                                    n * 128,
                                    cnt,
                                    2 * cfg.HID,
                                    single_packet=cfg.SP,
                                    queue_num=qn % 4,
                                )
                                qn += 1
                        ohE = opool.tile([128, nch, BW], bf16, tag="ohE")
                        ohO = opool.tile([128, nch, BW], bf16, tag="ohO")
                        if "oh" in ablate:
                            nc.vector.memset(ohE[:], 0.0)
                            nc.vector.memset(ohO[:], 0.0)
                        else:
                            io_b = (
                                iota_s[:, :]
                                .unsqueeze(1)
                                .broadcast_to([128, nch, BW])
                            )
                            rE_b = (
                                rrE_s[:, s0:s1]
                                .unsqueeze(2)
                                .broadcast_to([128, nch, BW])
                            )
                            rO_b = (
                                rrO_s[:, s0:s1]
                                .unsqueeze(2)
                                .broadcast_to([128, nch, BW])
                            )
                            nc.vector.tensor_tensor(ohE[:], io_b, rE_b, EQ)
                            nc.vector.tensor_tensor(ohO[:], io_b, rO_b, EQ)

                    rtile = None
                    if last:
                        rtile = resp.tile([BW, nb, cfg.HID], f32, tag="r")
                    for j, b in enumerate(blocks):
                        ps = psp.tile([BW, cfg.HID], f32, tag="ps")
                        bs0, bs1 = sched.block_slot_ranges[b]
                        k = 0
                        if "mm" not in ablate:
                            for s in range(bs0 - s0, bs1 - s0):
                                nc.tensor.matmul(
                                    ps[:],
                                    ohE[:, s, :],
                                    G[:, s, 0 : cfg.HID],
                                    start=(k == 0),
                                    stop=False,
                                )
                                k += 1
                                nc.tensor.matmul(
                                    ps[:],
                                    ohO[:, s, :],
                                    G[:, s, cfg.HID : 2 * cfg.HID],
                                    start=False,
                                    stop=False,
                                )
                                k += 1
                        # self-term: ps += I @ xs_cur[b]
                        nc.tensor.matmul(
                            ps[:],
                            I64_s[:],
                            xs_cur[:, b, :],
                            start=(k == 0),
                            stop=True,
                        )
                        nsrc = n1_s if last else n2_s
                        dst = rtile[:, j, :] if last else xs_nxt[:, b, :]
                        nc.scalar.activation(
                            dst, ps[:], COPY, scale=nsrc[:, b : b + 1]
                        )
                    if last:
                        store_group_to_dram(out_d, g, rtile[:])
                    else:
                        store_group_to_dram(
                            xs_in[l + 1], g, xs_nxt[:, b0 : b0 + nb, :]
                        )
                        if not no_ag and g in _chunk_end_group:
                            do_ag(l + 1, _chunk_end_group[g])

                if not last:
                    xs_cur, xs_nxt = xs_nxt, xs_cur

    nc.compile()
    return nc


def bench_bass(nc, in_maps, n_cores, iters=20, warmup=2):
    """Repeat-execution device benchmark (no NTFF tracing in this container).

    Mirrors bass2jax.run_bass_via_pjrt's multi-core path, minus output-buffer
    donation so the compiled executable can be re-invoked. Returns
    (results_list, per_iter_seconds).
    """
    import time

    import jax
    from jax.experimental.shard_map import shard_map
    from jax.sharding import Mesh, NamedSharding, PartitionSpec

    from concourse import bass2jax, mybir

    bass2jax.install_neuronx_cc_hook()

    partition_name = (
        nc.partition_id_tensor.name if nc.partition_id_tensor else None
    )
    in_names, out_names, out_avals, zero_outs = [], [], [], []
    for alloc in nc.m.functions[0].allocations:
        if not isinstance(alloc, mybir.MemoryLocationSet):
            continue
        name = alloc.memorylocations[0].name
        if alloc.kind == "ExternalInput":
            if name != partition_name:
                in_names.append(name)
        elif alloc.kind == "ExternalOutput":
            out_names.append(name)
            shape = tuple(alloc.tensor_shape)
            dtype = mybir.dt.np(alloc.dtype)
            out_avals.append(jax.core.ShapedArray(shape, dtype))
            zero_outs.append(np.zeros(shape, dtype))
    n_params = len(in_names)
    all_names = in_names + out_names
    if partition_name is not None:
        all_names = all_names + [partition_name]

    def _body(*args):
        operands = list(args)
        if partition_name is not None:
            operands.append(bass2jax.partition_id_tensor())
        outs = bass2jax._bass_exec_p.bind(
            *operands,
            out_avals=tuple(out_avals),
            in_names=tuple(all_names),
            out_names=tuple(out_names),
            lowering_input_output_aliases=(),
            sim_require_finite=True,
            sim_require_nnan=True,
            nc=nc,
        )
        return tuple(outs)

    devices = jax.devices()[:n_cores]
    mesh = Mesh(np.asarray(devices), ("core",))
    spec = PartitionSpec("core")
    nin = n_params + len(zero_outs)
    sharded = jax.jit(
        shard_map(
            _body,
            mesh=mesh,
            in_specs=(spec,) * nin,
            out_specs=(spec,) * len(out_names),
            check_rep=False,
        ),
        keep_unused=True,
    )
    sh = NamedSharding(mesh, spec)
    args = [
        jax.device_put(
            np.concatenate([np.asarray(m[name]) for m in in_maps], axis=0), sh
        )
        for name in in_names
    ] + [
        jax.device_put(
            np.zeros((n_cores * z.shape[0], *z.shape[1:]), z.dtype), sh
        )
        for z in zero_outs
    ]

    out_arrs = None
    for _ in range(warmup):
        out_arrs = sharded(*args)
        jax.block_until_ready(out_arrs)
    t0 = time.perf_counter()
    for _ in range(iters):
        out_arrs = sharded(*args)
    jax.block_until_ready(out_arrs)
    t1 = time.perf_counter()

    results = [
        {
            name: np.asarray(out_arrs[i]).reshape(n_cores, *out_avals[i].shape)[c]
            for i, name in enumerate(out_names)
        }
        for c in range(n_cores)
    ]
    return results, (t1 - t0) / iters


def kernel(x, edge_index, W, b, cfg: Cfg | None = None, trace: bool = False):
    """Full-input entry point. Returns [N, HID] float32 (+ results if trace)."""
    cfg = cfg or Cfg()
    sched, in_maps = host_inputs(x, edge_index, W, b, cfg)
    nc = build_bass(cfg, sched)

    from concourse import bass_utils

    res = bass_utils.run_bass_kernel_spmd(
        nc,
        in_maps,
        core_ids=list(range(cfg.CORES)),
        trace=False,
    )
    out = np.concatenate(
        [r["out"][: cfg.NPC] for r in res.results], axis=0
    )
    if trace:
        return out, res
    return out


if __name__ == "__main__":
    pass
